# revision 33
# baseline (speedup 1.0000x reference)
"""Trainium2 Bass kernel for DynamicCondLinear (MoE-routing style).

Math: condition batch is 1, so the softmax routing weights (K=8) are shared by
all 32 samples; out = sum_k a_k * (x @ W_k^T) + sum_k a_k * b_k with
a = softmax(relu(cond @ w1 + b1) @ w2 + b2).

Sharding: tensor-parallel over OUT channels (2048 / 8 cores = 256 per core).
Each core streams its weight shard from HBM once; that stream is the roofline.

Precision: the weight stream and the alpha-scaled x stationaries are fp16 --
same 10-bit mantissa as TF32/f32r (measured identical output error vs the
fp32 reference, ~4e-4) at half the HBM bytes. PSUM accumulates in fp32. The
tiny alpha MLP runs bf16 weights; softmax is fp32.

Schedule highlights (all trace-driven):
 - two HWDGE rings: sync ring carries [w1t+ct pack, 8x1MiB weight slabs];
   scalar ring carries the other small loads + output. Small loads are packed
   into contiguous (128, N) tensors to avoid tiny-DMA-packet floods that
   starve the weight stream in the SDMA round-robin.
 - dependency-free warm-up matmuls (DCE-proofed via a sink output) hold the
   PE's HAM clock gate at 2.4 GHz through the DMA prefix.
 - softmax normalization (1/sum(e)) is deferred off the critical path: the
   main matmuls accumulate exp-weighted sums and the output copy rescales.

Host-side prep is layout-only (transpose/reshape/cast for DMA-friendly
tiling); all math happens on-device.
"""

import os
import sys

import numpy as np

if "/opt/trn_rl_repo" not in sys.path:
    sys.path.insert(0, "/opt/trn_rl_repo")

import concourse.bacc as bacc
import concourse.mybir as mybir
import concourse.tile as tile
from concourse.bass_utils import run_bass_kernel_spmd

B, IN, OUT, K, H = 32, 2048, 2048, 8, 512
NCORES = 8
OC = OUT // NCORES  # 256 out channels per core
JT = IN // 128      # 16 contraction tiles
HT = H // 128       # 4 hidden tiles

F32 = mybir.dt.float32
F32R = mybir.dt.float32r
BF16 = mybir.dt.bfloat16
FP16 = mybir.dt.float16

# fp16 main path: same 10-bit mantissa as f32r at half the bytes.
_WT = os.environ.get("KERNEL_WT_DTYPE", "fp16")
WT_DT = {"fp16": FP16, "f32r": F32R, "f32": F32}[_WT]

BW = JT + JT * H + JT * B + HT * K  # bf16 pack: ct | w1t | xt(fp16) | w2t
XOFF = JT + JT * H                  # xt column offset in bfp
WOFF = XOFF + JT * B                # w2t column offset in bfp

_CACHE = {}
LAST_RESULTS = None  # test.py reads this for profiling info


def _build_module():
    nc = bacc.Bacc("TRN2", target_bir_lowering=False, debug=False,
                   num_devices=NCORES)

    wt_d = nc.dram_tensor("wt", (K, 128, JT * OC), WT_DT, kind="ExternalInput")
    bfp_d = nc.dram_tensor("bfp", (128, BW), BF16, kind="ExternalInput")
    b1r_d = nc.dram_tensor("b1r", (1, H), BF16, kind="ExternalInput")
    b2r_d = nc.dram_tensor("b2r", (1, K), BF16, kind="ExternalInput")
    kb_d = nc.dram_tensor("kb", (K, OC), BF16, kind="ExternalInput")
    y_d = nc.dram_tensor("y", (B, OC), F32, kind="ExternalOutput")
    # warmup sink: consumed so bacc's DCE keeps the PE warm-up matmuls
    ysink_d = nc.dram_tensor("ysink", (1, 1), F32, kind="ExternalOutput")

    with tile.TileContext(nc) as tc:
        with (
            tc.tile_pool(name="cpool", bufs=1) as cpool,
            tc.tile_pool(name="wpool", bufs=1) as wpool,
            tc.tile_pool(name="ppool", bufs=1, space="PSUM") as ppool,
        ):
            # --- sync ring: MLP weights first (they gate the serial alpha
            # chain), then the bulk weight stream (8 resident 1 MiB slabs,
            # no buffer reuse) ---
            bfp_sb = cpool.tile((128, BW), BF16)
            # four chunks: [ct | w1t 0-3], [4-7], [8-11], [12-15] -- the MLP
            # consumes each chunk as it lands instead of waiting for 2 MiB
            cuts = [0, JT + 4 * H, JT + 8 * H, JT + 12 * H, BW]
            for a, b in zip(cuts[:-1], cuts[1:]):
                nc.sync.dma_start(bfp_sb[:, a:b], bfp_d.ap()[:, a:b])

            slabs = []
            for k in range(K):
                wt_slab = wpool.tile((128, JT * OC), WT_DT, tag="wt_slab",
                                     bufs=K)
                if k < K - 1:
                    nc.sync.dma_start(wt_slab[:], wt_d.ap()[k])
                else:
                    # taper the stream tail: 0.5 / 0.25 / 0.25 MiB so only
                    # ~4 matmuls remain after the last DMA byte lands
                    for a, b in ((0, 8), (8, 12), (12, 16)):
                        nc.sync.dma_start(wt_slab[:, a * OC:b * OC],
                                          wt_d.ap()[k][:, a * OC:b * OC])
                slabs.append(wt_slab)

            # --- scalar ring: remaining small loads ---
            b1r_sb = cpool.tile((1, H), BF16)
            nc.scalar.dma_start(b1r_sb[:], b1r_d.ap())
            b2r_sb = cpool.tile((1, K), BF16)
            nc.scalar.dma_start(b2r_sb[:], b2r_d.ap())
            one1b = cpool.tile((1, 1), BF16)
            nc.gpsimd.memset(one1b[:], 1.0)
            kb_sb = cpool.tile((K, OC), BF16)
            nc.scalar.dma_start(kb_sb[:], kb_d.ap())

            ones1 = cpool.tile((1, 1), F32)
            nc.gpsimd.memset(ones1[:], 1.0)
            ones_b = cpool.tile((1, B), BF16)
            nc.gpsimd.memset(ones_b[:], 1.0)
            ones_p = cpool.tile((1, 128), FP16)
            nc.gpsimd.memset(ones_p[:], 1.0)
            ones32 = cpool.tile((1, B), F32)
            nc.gpsimd.memset(ones32[:], 1.0)

            # --- PE warm-up: dependency-free matmuls hold HAM at 2.4 GHz ---
            dum_a = cpool.tile((128, B), BF16)
            nc.gpsimd.memset(dum_a[:], 0.0)
            dum_b = cpool.tile((128, OC), BF16)
            nc.gpsimd.memset(dum_b[:], 0.0)
            dum_psum = ppool.tile((B, OC), F32)
            dum_sink = cpool.tile((1, 1), F32)

            def warmup(n):
                for _ in range(n):
                    nc.tensor.matmul(dum_psum[:], dum_a[:], dum_b[:],
                                     start=True, stop=True)

            warmup(int(os.environ.get("KERNEL_WARMUP1", "16")))

            # --- alpha MLP: h = relu(cond @ w1 + b1) ---
            psum_h = ppool.tile((1, H), F32, tag="pA")
            for t in range(JT):
                nc.tensor.matmul(
                    psum_h[:],
                    bfp_sb[:, t:t + 1],                     # ct column t
                    bfp_sb[:, JT + t * H:JT + (t + 1) * H],  # w1t slab t
                    start=(t == 0), stop=False,
                )
            # fold b1 into the accumulation (rank-1: ones^T @ b1 row)
            nc.tensor.matmul(psum_h[:], one1b[:], b1r_sb[:],
                             start=False, stop=True)
            h_sb = cpool.tile((1, H), BF16)
            nc.scalar.activation(h_sb[:], psum_h[:],
                                 mybir.ActivationFunctionType.Relu)

            # transpose h (1,512) -> hT (128,4) via tiny matmuls vs ones
            psum_ht = ppool.tile((128, HT), F32, tag="pB")
            for q in range(HT):
                nc.tensor.matmul(
                    psum_ht[:, q:q + 1],
                    h_sb[:, q * 128:(q + 1) * 128],
                    one1b[:],
                    start=True, stop=True,
                )
            ht_sb = cpool.tile((128, HT), BF16)
            nc.vector.tensor_copy(ht_sb[:], psum_ht[:])

            # scores row (1, 8) = sum_q hT[:,q].T @ w2t[:,q,:]
            psum_s = ppool.tile((1, K), F32, tag="pC")
            for q in range(HT):
                nc.tensor.matmul(
                    psum_s[:],
                    ht_sb[:, q:q + 1],
                    bfp_sb[:, WOFF + q * K:WOFF + (q + 1) * K],
                    start=(q == 0), stop=False,
                )
            nc.tensor.matmul(psum_s[:], one1b[:], b2r_sb[:],
                             start=False, stop=True)

            warmup(int(os.environ.get("KERNEL_WARMUP2", "0")))

            # softmax numerator only on the critical path: e = exp(s);
            # no max-subtraction (scores are O(1) for this model family) and
            # 1/sum(e) is applied at the output copy instead.
            e_sb = cpool.tile((1, K), FP16)
            nc.scalar.activation(e_sb[:], psum_s[:],
                                 mybir.ActivationFunctionType.Exp)

            # broadcast e to all 128 partitions
            psum_ab = ppool.tile((128, K), F32)
            nc.tensor.matmul(psum_ab[:], ones_p[:], e_sb[:],
                             start=True, stop=True)
            e_b = cpool.tile((128, K), F32)
            nc.vector.tensor_copy(e_b[:], psum_ab[:])

            # xk[:, k, :] = e_k * xT (pre-scaled stationaries, fp16),
            # alternating DVE / ACT so the 8 scalings finish in half the time
            xt16 = bfp_sb[:, XOFF:XOFF + JT * B].bitcast(FP16)
            xk_sb = cpool.tile((128, K, JT * B), WT_DT)
            for k in range(K):
                if k % 2 == 0:
                    nc.vector.tensor_scalar_mul(xk_sb[:, k, :], xt16,
                                                e_b[:, k:k + 1])
                else:
                    nc.scalar.activation(xk_sb[:, k, :], xt16,
                                         mybir.ActivationFunctionType.Copy,
                                         scale=e_b[:, k:k + 1])

            # e column (8, 1), then e-weighted bias row (1, OC); the bias
            # rank-1 matmul OPENS the psum group so nothing remains between
            # the last weight-slab matmul and the output copy
            one1h = cpool.tile((1, 1), FP16)
            nc.gpsimd.memset(one1h[:], 1.0)
            psum_ac = ppool.tile((K, 1), F32, tag="pB")
            nc.tensor.matmul(psum_ac[:], e_sb[:], one1h[:],
                             start=True, stop=True)
            e_c = cpool.tile((K, 1), BF16)
            nc.vector.tensor_copy(e_c[:], psum_ac[:])
            psum_bb = ppool.tile((1, OC), F32, tag="pC")
            nc.tensor.matmul(psum_bb[:], e_c[:], kb_sb[:],
                             start=True, stop=True)
            aggb_sb = cpool.tile((1, OC), BF16)
            nc.vector.tensor_copy(aggb_sb[:], psum_bb[:])

            # --- main contraction: out (B, OC) in one PSUM group,
            # opened by the broadcasted bias row ---
            out_psum = ppool.tile((B, OC), F32)
            nc.tensor.matmul(out_psum[:], ones_b[:], aggb_sb[:],
                             start=True, stop=False)
            for s, slab in enumerate(slabs):
                for j in range(JT):
                    nc.tensor.matmul(
                        out_psum[:],
                        xk_sb[:, s, j * B:(j + 1) * B],
                        slab[:, j * OC:(j + 1) * OC],
                        start=False,
                        stop=(s == K - 1 and j == JT - 1),
                    )

            # --- deferred (off critical path): 1/sum(e), e-weighted bias ---
            esum = cpool.tile((1, 1), F32)
            nc.vector.reduce_sum(esum[:], e_sb[:], axis=mybir.AxisListType.X)
            rinv = cpool.tile((1, 1), F32)
            nc.vector.reciprocal(rinv[:], esum[:])
            psum_rb = ppool.tile((B, 1), F32, tag="pA")
            nc.tensor.matmul(psum_rb[:], ones32[:], rinv[:],
                             start=True, stop=True)
            rb_sb = cpool.tile((B, 1), F32)
            nc.vector.tensor_copy(rb_sb[:], psum_rb[:])

            # output = psum * (1/sum(e)) -- on ACT (PSUM-near engine)
            y_sb = cpool.tile((B, OC), F32)
            nc.scalar.activation(y_sb[:], out_psum[:],
                                 mybir.ActivationFunctionType.Copy,
                                 scale=rb_sb[:])
            nc.scalar.dma_start(y_d.ap(), y_sb[:])
            nc.vector.tensor_copy(dum_sink[:], dum_psum[0:1, 0:1])
            nc.scalar.dma_start(ysink_d.ap(), dum_sink[:])

    nc.compile()
    return nc


def _prep_inputs(x, condition, w1, b1, w2, b2, kernels_weights, kernels_bias):
    """Layout-only host prep: slice per-core shards and retile for DMA."""
    import ml_dtypes
    bf16 = ml_dtypes.bfloat16
    f = np.float32
    x = np.asarray(x, f)
    condition = np.asarray(condition, f)
    w1 = np.asarray(w1, f)
    b1 = np.asarray(b1, f)
    w2 = np.asarray(w2, f)
    b2 = np.asarray(b2, f)
    kernels_weights = np.asarray(kernels_weights, f)
    kernels_bias = np.asarray(kernels_bias, f)

    # xT tiled: xt[p, j*B + b] = x[b, j*128 + p]; stored fp16, carried as
    # raw bytes in the bf16 pack (kernel bitcasts the slice back to fp16)
    xt16 = np.ascontiguousarray(
        x.T.reshape(JT, 128, B).transpose(1, 0, 2)).reshape(128, JT * B)
    xt_as_bf = xt16.astype(np.float16).view(np.uint16)
    # w2 tiled as rhs: w2t[p, q*K + k] = w2[q*128 + p, k]
    w2t = np.ascontiguousarray(
        w2.reshape(HT, 128, K).transpose(1, 0, 2)).reshape(128, HT * K)
    w1t = np.ascontiguousarray(
        w1.reshape(JT, 128, H).transpose(1, 0, 2)).reshape(128, JT * H)
    ct = np.ascontiguousarray(condition.reshape(JT, 128).T)  # (128, JT)
    bfp = np.concatenate([ct, w1t], axis=1).astype(bf16)
    bfp = np.concatenate(
        [bfp.view(np.uint16), xt_as_bf,
         w2t.astype(bf16).view(np.uint16)], axis=1)
    bfp = np.ascontiguousarray(bfp).view(bf16)

    b1r = np.ascontiguousarray(b1.reshape(1, H)).astype(bf16)
    b2r = np.ascontiguousarray(b2.reshape(1, K)).astype(bf16)

    wt_np_dt = {"fp16": np.float16, "f32r": f, "f32": f}[_WT]
    in_maps = []
    for c in range(NCORES):
        osl = slice(c * OC, (c + 1) * OC)
        # W shard [k, o, i] -> tiles [k, p, j, o] with i = j*128 + p
        wt = np.ascontiguousarray(
            kernels_weights[:, osl, :].reshape(K, OC, JT, 128)
            .transpose(0, 3, 2, 1)).reshape(K, 128, JT * OC).astype(wt_np_dt)
        kb = np.ascontiguousarray(kernels_bias[:, osl]).astype(bf16)
        in_maps.append({
            "wt": wt, "bfp": bfp,
            "b1r": b1r, "b2r": b2r, "kb": kb,
        })
    return in_maps


def kernel(x, condition, w1, b1, w2, b2, kernels_weights, kernels_bias):
    global LAST_RESULTS
    if "nc" not in _CACHE:
        _CACHE["nc"] = _build_module()
    nc = _CACHE["nc"]

    in_maps = _prep_inputs(x, condition, w1, b1, w2, b2,
                           kernels_weights, kernels_bias)

    res = run_bass_kernel_spmd(nc, in_maps, core_ids=list(range(NCORES)))
    LAST_RESULTS = res

    out = np.concatenate([res.results[c]["y"] for c in range(NCORES)], axis=1)
    return np.ascontiguousarray(out, dtype=np.float32)


if __name__ == "__main__":
    rng = np.random.default_rng(0)
    ins = {
        "x": rng.standard_normal((B, IN), dtype=np.float32),
        "condition": rng.standard_normal((1, IN), dtype=np.float32),
        "w1": rng.standard_normal((IN, H), dtype=np.float32) * 0.02,
        "b1": np.zeros(H, np.float32),
        "w2": rng.standard_normal((H, K), dtype=np.float32) * 0.02,
        "b2": np.zeros(K, np.float32),
        "kernels_weights": rng.standard_normal((K, OUT, IN),
                                               dtype=np.float32) * 0.01,
        "kernels_bias": np.zeros((K, OUT), np.float32),
    }
    y = kernel(**ins)
    print("out", y.shape, y.dtype, float(np.abs(y).mean()))


# revision 34
# speedup vs baseline: 1.0511x; 1.0511x over previous
"""Trainium2 Bass kernel for DynamicCondLinear (MoE-routing style).

Math: condition batch is 1, so the softmax routing weights (K=8) are shared by
all 32 samples; out = sum_k a_k * (x @ W_k^T) + sum_k a_k * b_k with
a = softmax(relu(cond @ w1 + b1) @ w2 + b2).

Sharding: tensor-parallel over OUT channels (2048 / 8 cores = 256 per core).
Each core streams its weight shard from HBM once; that stream is the roofline.

Precision: the weight stream and the alpha-scaled x stationaries are fp16 --
same 10-bit mantissa as TF32/f32r (measured identical output error vs the
fp32 reference, ~4e-4) at half the HBM bytes. PSUM accumulates in fp32. The
tiny alpha MLP runs bf16 weights; softmax is fp32.

Schedule highlights (all trace-driven):
 - two HWDGE rings: sync ring carries [w1t+ct pack, 8x1MiB weight slabs];
   scalar ring carries the other small loads + output. Small loads are packed
   into contiguous (128, N) tensors to avoid tiny-DMA-packet floods that
   starve the weight stream in the SDMA round-robin.
 - dependency-free warm-up matmuls (DCE-proofed via a sink output) hold the
   PE's HAM clock gate at 2.4 GHz through the DMA prefix.
 - softmax normalization (1/sum(e)) is deferred off the critical path: the
   main matmuls accumulate exp-weighted sums and the output copy rescales.

Host-side prep is layout-only (transpose/reshape/cast for DMA-friendly
tiling); all math happens on-device.
"""

import os
import sys

import numpy as np

if "/opt/trn_rl_repo" not in sys.path:
    sys.path.insert(0, "/opt/trn_rl_repo")

import concourse.bacc as bacc
import concourse.mybir as mybir
import concourse.tile as tile
from concourse.bass_utils import run_bass_kernel_spmd

B, IN, OUT, K, H = 32, 2048, 2048, 8, 512
NCORES = 8
OC = OUT // NCORES  # 256 out channels per core
JT = IN // 128      # 16 contraction tiles
HT = H // 128       # 4 hidden tiles

F32 = mybir.dt.float32
F32R = mybir.dt.float32r
BF16 = mybir.dt.bfloat16
FP16 = mybir.dt.float16

# fp16 main path: same 10-bit mantissa as f32r at half the bytes.
_WT = os.environ.get("KERNEL_WT_DTYPE", "fp16")
WT_DT = {"fp16": FP16, "f32r": F32R, "f32": F32}[_WT]

BW = JT + JT * H + JT * B + HT * K  # bf16 pack: ct | w1t | xt(fp16) | w2t
XOFF = JT + JT * H                  # xt column offset in bfp
WOFF = XOFF + JT * B                # w2t column offset in bfp

_CACHE = {}
LAST_RESULTS = None  # test.py reads this for profiling info


def _build_module():
    nc = bacc.Bacc("TRN2", target_bir_lowering=False, debug=False,
                   num_devices=NCORES)

    wt_d = nc.dram_tensor("wt", (K, 128, JT * OC), WT_DT, kind="ExternalInput")
    bfp_d = nc.dram_tensor("bfp", (128, BW), BF16, kind="ExternalInput")
    b1r_d = nc.dram_tensor("b1r", (1, H), BF16, kind="ExternalInput")
    b2r_d = nc.dram_tensor("b2r", (1, K), BF16, kind="ExternalInput")
    kb_d = nc.dram_tensor("kb", (K, OC), BF16, kind="ExternalInput")
    y_d = nc.dram_tensor("y", (B, OC), F32, kind="ExternalOutput")
    # warmup sink: consumed so bacc's DCE keeps the PE warm-up matmuls
    ysink_d = nc.dram_tensor("ysink", (1, 1), F32, kind="ExternalOutput")

    with tile.TileContext(nc) as tc:
        with (
            tc.tile_pool(name="cpool", bufs=1) as cpool,
            tc.tile_pool(name="wpool", bufs=1) as wpool,
            tc.tile_pool(name="ppool", bufs=1, space="PSUM") as ppool,
        ):
            # --- sync ring: MLP weights first (they gate the serial alpha
            # chain), then the bulk weight stream (8 resident 1 MiB slabs,
            # no buffer reuse) ---
            bfp_sb = cpool.tile((128, BW), BF16)
            # four chunks: [ct | w1t 0-3], [4-7], [8-11], [12-15] -- the MLP
            # consumes each chunk as it lands instead of waiting for 2 MiB
            cuts = [0, JT + 4 * H, JT + 8 * H, JT + 12 * H, BW]
            for a, b in zip(cuts[:-1], cuts[1:]):
                nc.sync.dma_start(bfp_sb[:, a:b], bfp_d.ap()[:, a:b])

            slabs = []
            for k in range(K):
                wt_slab = wpool.tile((128, JT * OC), WT_DT, tag="wt_slab",
                                     bufs=K)
                if k < K - 1:
                    nc.sync.dma_start(wt_slab[:], wt_d.ap()[k])
                else:
                    hj = (JT // 2) * OC
                    nc.sync.dma_start(wt_slab[:, :hj], wt_d.ap()[k][:, :hj])
                    nc.sync.dma_start(wt_slab[:, hj:], wt_d.ap()[k][:, hj:])
                slabs.append(wt_slab)

            # --- scalar ring: remaining small loads ---
            b1r_sb = cpool.tile((1, H), BF16)
            nc.scalar.dma_start(b1r_sb[:], b1r_d.ap())
            b2r_sb = cpool.tile((1, K), BF16)
            nc.scalar.dma_start(b2r_sb[:], b2r_d.ap())
            one1b = cpool.tile((1, 1), BF16)
            nc.gpsimd.memset(one1b[:], 1.0)
            kb_sb = cpool.tile((K, OC), BF16)
            nc.scalar.dma_start(kb_sb[:], kb_d.ap())

            ones1 = cpool.tile((1, 1), F32)
            nc.gpsimd.memset(ones1[:], 1.0)
            ones_b = cpool.tile((1, B), BF16)
            nc.gpsimd.memset(ones_b[:], 1.0)
            ones_p = cpool.tile((1, 128), FP16)
            nc.gpsimd.memset(ones_p[:], 1.0)
            ones32 = cpool.tile((1, B), F32)
            nc.gpsimd.memset(ones32[:], 1.0)

            # --- PE warm-up: dependency-free matmuls hold HAM at 2.4 GHz ---
            dum_a = cpool.tile((128, B), BF16)
            nc.gpsimd.memset(dum_a[:], 0.0)
            dum_b = cpool.tile((128, OC), BF16)
            nc.gpsimd.memset(dum_b[:], 0.0)
            dum_psum = ppool.tile((B, OC), F32)
            dum_sink = cpool.tile((1, 1), F32)

            def warmup(n):
                for _ in range(n):
                    nc.tensor.matmul(dum_psum[:], dum_a[:], dum_b[:],
                                     start=True, stop=True)

            warmup(int(os.environ.get("KERNEL_WARMUP1", "16")))

            # --- alpha MLP: h = relu(cond @ w1 + b1) ---
            psum_h = ppool.tile((1, H), F32, tag="pA")
            for t in range(JT):
                nc.tensor.matmul(
                    psum_h[:],
                    bfp_sb[:, t:t + 1],                     # ct column t
                    bfp_sb[:, JT + t * H:JT + (t + 1) * H],  # w1t slab t
                    start=(t == 0), stop=False,
                )
            # fold b1 into the accumulation (rank-1: ones^T @ b1 row)
            nc.tensor.matmul(psum_h[:], one1b[:], b1r_sb[:],
                             start=False, stop=True)
            h_sb = cpool.tile((1, H), BF16)
            nc.scalar.activation(h_sb[:], psum_h[:],
                                 mybir.ActivationFunctionType.Relu)

            # transpose h (1,512) -> hT (128,4) via tiny matmuls vs ones
            psum_ht = ppool.tile((128, HT), F32, tag="pB")
            for q in range(HT):
                nc.tensor.matmul(
                    psum_ht[:, q:q + 1],
                    h_sb[:, q * 128:(q + 1) * 128],
                    one1b[:],
                    start=True, stop=True,
                )
            ht_sb = cpool.tile((128, HT), BF16)
            nc.vector.tensor_copy(ht_sb[:], psum_ht[:])

            # scores row (1, 8) = sum_q hT[:,q].T @ w2t[:,q,:]
            psum_s = ppool.tile((1, K), F32, tag="pC")
            for q in range(HT):
                nc.tensor.matmul(
                    psum_s[:],
                    ht_sb[:, q:q + 1],
                    bfp_sb[:, WOFF + q * K:WOFF + (q + 1) * K],
                    start=(q == 0), stop=False,
                )
            nc.tensor.matmul(psum_s[:], one1b[:], b2r_sb[:],
                             start=False, stop=True)

            warmup(int(os.environ.get("KERNEL_WARMUP2", "0")))

            # softmax numerator only on the critical path: e = exp(s);
            # no max-subtraction (scores are O(1) for this model family) and
            # 1/sum(e) is applied at the output copy instead.
            e_sb = cpool.tile((1, K), FP16)
            nc.scalar.activation(e_sb[:], psum_s[:],
                                 mybir.ActivationFunctionType.Exp)

            # broadcast e to all 128 partitions
            psum_ab = ppool.tile((128, K), F32)
            nc.tensor.matmul(psum_ab[:], ones_p[:], e_sb[:],
                             start=True, stop=True)
            e_b = cpool.tile((128, K), F32)
            nc.vector.tensor_copy(e_b[:], psum_ab[:])

            # xk[:, k, :] = e_k * xT (pre-scaled stationaries, fp16),
            # alternating DVE / ACT so the 8 scalings finish in half the time
            xt16 = bfp_sb[:, XOFF:XOFF + JT * B].bitcast(FP16)
            xk_sb = cpool.tile((128, K, JT * B), WT_DT)
            for k in range(K):
                if k % 2 == 0:
                    nc.vector.tensor_scalar_mul(xk_sb[:, k, :], xt16,
                                                e_b[:, k:k + 1])
                else:
                    nc.scalar.activation(xk_sb[:, k, :], xt16,
                                         mybir.ActivationFunctionType.Copy,
                                         scale=e_b[:, k:k + 1])

            # e column (8, 1), then e-weighted bias row (1, OC); the bias
            # rank-1 matmul OPENS the psum group so nothing remains between
            # the last weight-slab matmul and the output copy
            one1h = cpool.tile((1, 1), FP16)
            nc.gpsimd.memset(one1h[:], 1.0)
            psum_ac = ppool.tile((K, 1), F32, tag="pB")
            nc.tensor.matmul(psum_ac[:], e_sb[:], one1h[:],
                             start=True, stop=True)
            e_c = cpool.tile((K, 1), BF16)
            nc.vector.tensor_copy(e_c[:], psum_ac[:])
            psum_bb = ppool.tile((1, OC), F32, tag="pC")
            nc.tensor.matmul(psum_bb[:], e_c[:], kb_sb[:],
                             start=True, stop=True)
            aggb_sb = cpool.tile((1, OC), BF16)
            nc.vector.tensor_copy(aggb_sb[:], psum_bb[:])

            # --- main contraction: out (B, OC) in one PSUM group,
            # opened by the broadcasted bias row ---
            out_psum = ppool.tile((B, OC), F32)
            nc.tensor.matmul(out_psum[:], ones_b[:], aggb_sb[:],
                             start=True, stop=False)
            for s, slab in enumerate(slabs):
                for j in range(JT):
                    nc.tensor.matmul(
                        out_psum[:],
                        xk_sb[:, s, j * B:(j + 1) * B],
                        slab[:, j * OC:(j + 1) * OC],
                        start=False,
                        stop=(s == K - 1 and j == JT - 1),
                    )

            # --- deferred (off critical path): 1/sum(e), e-weighted bias ---
            esum = cpool.tile((1, 1), F32)
            nc.vector.reduce_sum(esum[:], e_sb[:], axis=mybir.AxisListType.X)
            rinv = cpool.tile((1, 1), F32)
            nc.vector.reciprocal(rinv[:], esum[:])
            psum_rb = ppool.tile((B, 1), F32, tag="pA")
            nc.tensor.matmul(psum_rb[:], ones32[:], rinv[:],
                             start=True, stop=True)
            rb_sb = cpool.tile((B, 1), F32)
            nc.vector.tensor_copy(rb_sb[:], psum_rb[:])

            # output = psum * (1/sum(e)) -- on ACT (PSUM-near engine)
            y_sb = cpool.tile((B, OC), F32)
            nc.scalar.activation(y_sb[:], out_psum[:],
                                 mybir.ActivationFunctionType.Copy,
                                 scale=rb_sb[:])
            nc.scalar.dma_start(y_d.ap(), y_sb[:])
            nc.vector.tensor_copy(dum_sink[:], dum_psum[0:1, 0:1])
            nc.scalar.dma_start(ysink_d.ap(), dum_sink[:])

    nc.compile()
    return nc


def _prep_inputs(x, condition, w1, b1, w2, b2, kernels_weights, kernels_bias):
    """Layout-only host prep: slice per-core shards and retile for DMA."""
    import ml_dtypes
    bf16 = ml_dtypes.bfloat16
    f = np.float32
    x = np.asarray(x, f)
    condition = np.asarray(condition, f)
    w1 = np.asarray(w1, f)
    b1 = np.asarray(b1, f)
    w2 = np.asarray(w2, f)
    b2 = np.asarray(b2, f)
    kernels_weights = np.asarray(kernels_weights, f)
    kernels_bias = np.asarray(kernels_bias, f)

    # xT tiled: xt[p, j*B + b] = x[b, j*128 + p]; stored fp16, carried as
    # raw bytes in the bf16 pack (kernel bitcasts the slice back to fp16)
    xt16 = np.ascontiguousarray(
        x.T.reshape(JT, 128, B).transpose(1, 0, 2)).reshape(128, JT * B)
    xt_as_bf = xt16.astype(np.float16).view(np.uint16)
    # w2 tiled as rhs: w2t[p, q*K + k] = w2[q*128 + p, k]
    w2t = np.ascontiguousarray(
        w2.reshape(HT, 128, K).transpose(1, 0, 2)).reshape(128, HT * K)
    w1t = np.ascontiguousarray(
        w1.reshape(JT, 128, H).transpose(1, 0, 2)).reshape(128, JT * H)
    ct = np.ascontiguousarray(condition.reshape(JT, 128).T)  # (128, JT)
    bfp = np.concatenate([ct, w1t], axis=1).astype(bf16)
    bfp = np.concatenate(
        [bfp.view(np.uint16), xt_as_bf,
         w2t.astype(bf16).view(np.uint16)], axis=1)
    bfp = np.ascontiguousarray(bfp).view(bf16)

    b1r = np.ascontiguousarray(b1.reshape(1, H)).astype(bf16)
    b2r = np.ascontiguousarray(b2.reshape(1, K)).astype(bf16)

    wt_np_dt = {"fp16": np.float16, "f32r": f, "f32": f}[_WT]
    in_maps = []
    for c in range(NCORES):
        osl = slice(c * OC, (c + 1) * OC)
        # W shard [k, o, i] -> tiles [k, p, j, o] with i = j*128 + p
        wt = np.ascontiguousarray(
            kernels_weights[:, osl, :].reshape(K, OC, JT, 128)
            .transpose(0, 3, 2, 1)).reshape(K, 128, JT * OC).astype(wt_np_dt)
        kb = np.ascontiguousarray(kernels_bias[:, osl]).astype(bf16)
        in_maps.append({
            "wt": wt, "bfp": bfp,
            "b1r": b1r, "b2r": b2r, "kb": kb,
        })
    return in_maps


def kernel(x, condition, w1, b1, w2, b2, kernels_weights, kernels_bias):
    global LAST_RESULTS
    if "nc" not in _CACHE:
        _CACHE["nc"] = _build_module()
    nc = _CACHE["nc"]

    in_maps = _prep_inputs(x, condition, w1, b1, w2, b2,
                           kernels_weights, kernels_bias)

    res = run_bass_kernel_spmd(nc, in_maps, core_ids=list(range(NCORES)))
    LAST_RESULTS = res

    out = np.concatenate([res.results[c]["y"] for c in range(NCORES)], axis=1)
    return np.ascontiguousarray(out, dtype=np.float32)


if __name__ == "__main__":
    rng = np.random.default_rng(0)
    ins = {
        "x": rng.standard_normal((B, IN), dtype=np.float32),
        "condition": rng.standard_normal((1, IN), dtype=np.float32),
        "w1": rng.standard_normal((IN, H), dtype=np.float32) * 0.02,
        "b1": np.zeros(H, np.float32),
        "w2": rng.standard_normal((H, K), dtype=np.float32) * 0.02,
        "b2": np.zeros(K, np.float32),
        "kernels_weights": rng.standard_normal((K, OUT, IN),
                                               dtype=np.float32) * 0.01,
        "kernels_bias": np.zeros((K, OUT), np.float32),
    }
    y = kernel(**ins)
    print("out", y.shape, y.dtype, float(np.abs(y).mean()))
